# revision 4
# baseline (speedup 1.0000x reference)
"""Trainium2 Bass kernel for nn_ConnectionC2G (GNN cross-attention message passing).

Math (per batch b):
    K = Wk @ img + bk            [32, L]   (img = image reshaped [256, L], L = 4096)
    V = Wv @ img + bv            [32, L]
    Qt = (Wq @ graph^T + bq)/s   [32, N]   (s = sqrt(32); scale folded into Wq, bq)
    S^T[l, n] = sum_o K[o,l] Qt[o,n]       (attention scores, transposed layout)
    softmax over n-axis of the ORIGINAL layout == per-l-row softmax in S^T layout
    message[o, n] = sum_l (V[o,l]/den[l]) * exp(S^T[l,n])
    out^T = graph^T + Wc @ message + bc    [32, N]

Key tricks:
  - scores lie in [-2.6, 2.7] for this problem so exp() never overflows ->
    no max-subtraction pass; ScalarE activation computes exp straight from
    PSUM and its accum_out gives the softmax denominator for free.
  - 1/den is folded into V^T columns (per-partition scalar multiply) instead
    of normalizing the big [L, N] matrix.
  - message accumulates across all 32 l-tiles into 2 persistent PSUM banks
    using tile_position column strips (M=32 outputs packed 4-per-bank).
  - sharding: data-parallel over batch, 1 batch per NeuronCore (8 cores).

Host side pre-transposes graph -> graph^T, converts the image to bf16 in a
[128, 2*L] channel-split layout, packs the tiny weights, and transposes the
[32, N] device output back to [N, 32].
"""

import numpy as np
import ml_dtypes

import concourse.bass as bass
import concourse.bacc as bacc
import concourse.tile as tile
from concourse import mybir, masks
from concourse.bass_utils import run_bass_kernel_spmd

F32 = mybir.dt.float32
BF16 = mybir.dt.bfloat16
AF = mybir.ActivationFunctionType
OP = mybir.AluOpType

B = 8
N = 4096          # graph nodes
GC = 32           # graph channels
C = 256           # image channels
L = 4096          # image pixels (64*64)
LT = 128          # l-tile rows (partition dim of S^T tiles)
NLT = L // LT     # 32 l-tiles
NB = 512          # matmul moving-dim block
NNB = N // NB     # 8 n-blocks
# exp chunk boundaries within an l-tile's 4096 n-columns (3 PSUM banks each)
CHUNKS = [(0, 1536), (1536, 3072), (3072, 4096)]

TRACE = False            # test.py sets kernel.TRACE = True for profiling
LAST_RESULT = None       # test.py reads exec_time_ns from here

_NC_CACHE = {}


def build_kernel():
    nc = bacc.Bacc("TRN2")

    img_d = nc.dram_tensor("img", [128, 2 * L], BF16, kind="ExternalInput")
    graphT_d = nc.dram_tensor("graphT", [GC, N], F32, kind="ExternalInput")
    # bf16 pack: [:,0:32] WkT rows 0:128 | [:,32:64] WkT rows 128:256
    #            [:,64:96] WvT rows 0:128 | [:,96:128] WvT rows 128:256
    #            [0:32,128:160] WcT
    wkv_d = nc.dram_tensor("wkv", [128, 160], BF16, kind="ExternalInput")
    # f32 pack: [:,0:32] WqT*s | [:,32] bq*s | [:,33] bk | [:,34] bv | [:,35] bc
    wq_d = nc.dram_tensor("wq", [GC, 40], F32, kind="ExternalInput")
    out_d = nc.dram_tensor("outT", [GC, N], F32, kind="ExternalOutput")

    with tile.TileContext(nc) as tc:
        with tc.tile_pool(name="persist", bufs=1) as persist:
            img = persist.tile([128, 2 * L], BF16, tag="img")
            graphT = persist.tile([GC, N], F32, tag="graphT")
            wkv = persist.tile([128, 160], BF16, tag="wkv")
            wq = persist.tile([GC, 40], F32, tag="wq")
            K_sb = persist.tile([GC, N], BF16, tag="K_sb")
            Qt = persist.tile([GC, N], BF16, tag="Qt")
            V_sb = persist.tile([GC, N], F32, tag="V_sb")
            Vt_raw = persist.tile([128, NLT * GC], F32, tag="Vt_raw")
            ident = persist.tile([GC, GC], F32, tag="ident")
            msg_sb = persist.tile([GC, N], BF16, tag="msg_sb")
            outT = persist.tile([GC, N], F32, tag="outT")

            nc.sync.dma_start(out=img[:], in_=img_d[:])
            nc.sync.dma_start(out=graphT[:], in_=graphT_d[:])
            nc.sync.dma_start(out=wkv[:], in_=wkv_d[:])
            nc.sync.dma_start(out=wq[:], in_=wq_d[:])
            masks.make_identity(nc, ident[:])

            bq = wq[:, 32:33]
            bk = wq[:, 33:34]
            bv = wq[:, 34:35]
            bc = wq[:, 35:36]

            # ---- prologue: K/V/Q projections + V transposes --------------
            with (
                tc.tile_pool(name="proj_psum", bufs=3,
                             space=bass.MemorySpace.PSUM) as pp,
                tc.tile_pool(name="vt_psum", bufs=2,
                             space=bass.MemorySpace.PSUM) as vtp,
            ):
                for j in range(NNB):
                    blk = slice(j * NB, (j + 1) * NB)
                    kp = pp.tile([GC, NB], F32, tag="proj")
                    nc.tensor.matmul(kp[:], wkv[:, 0:32], img[:, blk],
                                     start=True, stop=False)
                    nc.tensor.matmul(kp[:], wkv[:, 32:64],
                                     img[:, L + j * NB:L + (j + 1) * NB],
                                     start=False, stop=True)
                    nc.vector.tensor_scalar_add(K_sb[:, blk], kp[:], bk)

                    vp = pp.tile([GC, NB], F32, tag="proj")
                    nc.tensor.matmul(vp[:], wkv[:, 64:96], img[:, blk],
                                     start=True, stop=False)
                    nc.tensor.matmul(vp[:], wkv[:, 96:128],
                                     img[:, L + j * NB:L + (j + 1) * NB],
                                     start=False, stop=True)
                    nc.vector.tensor_scalar_add(V_sb[:, blk], vp[:], bv)

                    qp = pp.tile([GC, NB], F32, tag="proj")
                    nc.tensor.matmul(qp[:], wq[:, 0:32], graphT[:, blk],
                                     start=True, stop=True)
                    nc.vector.tensor_scalar_add(Qt[:, blk], qp[:], bq)

                    # transpose the 4 V l-tiles living in this 512-col block
                    for t in range(4):
                        lt = j * 4 + t
                        vt = vtp.tile([128, GC], F32, tag="vt")
                        nc.tensor.transpose(
                            vt[:], V_sb[:, lt * LT:(lt + 1) * LT], ident[:])
                        nc.vector.tensor_copy(
                            Vt_raw[:, lt * GC:(lt + 1) * GC], vt[:])

            # ---- main loop: scores -> exp -> message ---------------------
            with (
                tc.tile_pool(name="s_psum", bufs=2,
                             space=bass.MemorySpace.PSUM) as sp,
                tc.tile_pool(name="msg_psum", bufs=1,
                             space=bass.MemorySpace.PSUM) as mp,
                tc.tile_pool(name="e_pool", bufs=2) as ep,
                tc.tile_pool(name="stat", bufs=6) as stp,
            ):
                msg_ps = mp.tile([128, 1024], F32, tag="msg")
                for lt in range(NLT):
                    k_station = K_sb[:, lt * LT:(lt + 1) * LT]
                    e_t = ep.tile([128, N], BF16, tag="E")
                    accs = []
                    for (c0, c1) in CHUNKS:
                        w = c1 - c0
                        s_t = sp.tile([128, 1536], F32, tag="S")
                        for m in range(w // NB):
                            nc.tensor.matmul(
                                s_t[:, m * NB:(m + 1) * NB],
                                k_station,
                                Qt[:, c0 + m * NB:c0 + (m + 1) * NB],
                                start=True, stop=True)
                        acc = stp.tile([128, 1], F32, tag=f"acc{c0}")
                        nc.scalar.activation(
                            out=e_t[:, c0:c1], in_=s_t[:, 0:w],
                            func=AF.Exp, accum_out=acc[:])
                        accs.append(acc)
                    den = stp.tile([128, 1], F32, tag="den")
                    nc.vector.scalar_tensor_tensor(
                        out=den[:], in0=accs[0][:], scalar=accs[1][:],
                        in1=accs[2][:], op0=OP.add, op1=OP.add)
                    rden = stp.tile([128, 1], F32, tag="rden")
                    nc.vector.reciprocal(rden[:], den[:])
                    vts = stp.tile([128, GC], BF16, tag="vts")
                    nc.vector.tensor_scalar_mul(
                        vts[:], Vt_raw[:, lt * GC:(lt + 1) * GC], rden[:])
                    for j in range(NNB):
                        cg = 32 * (j % 4)
                        hb = (j // 4) * NB
                        nc.tensor.matmul(
                            msg_ps[cg:cg + 32, hb:hb + NB],
                            vts[:], e_t[:, j * NB:(j + 1) * NB],
                            start=(lt == 0), stop=(lt == NLT - 1),
                            tile_position=(0, cg))

                # unpack message strips to SBUF while pools still own psum
                for j in range(NNB):
                    cg = 32 * (j % 4)
                    hb = (j // 4) * NB
                    nc.vector.tensor_copy(
                        msg_sb[:, j * NB:(j + 1) * NB],
                        msg_ps[cg:cg + 32, hb:hb + NB])

            # ---- tail: Wc projection + residual --------------------------
            with tc.tile_pool(name="tail_psum", bufs=2,
                              space=bass.MemorySpace.PSUM) as tp:
                for j in range(NNB):
                    blk = slice(j * NB, (j + 1) * NB)
                    pj = tp.tile([GC, NB], F32, tag="prj")
                    nc.tensor.matmul(pj[:], wkv[0:32, 128:160], msg_sb[:, blk],
                                     start=True, stop=True)
                    nc.vector.scalar_tensor_tensor(
                        out=outT[:, blk], in0=pj[:], scalar=bc,
                        in1=graphT[:, blk], op0=OP.add, op1=OP.add)
                nc.sync.dma_start(out=out_d[:], in_=outT[:])

    nc.finalize()
    return nc


def _get_nc():
    if "nc" not in _NC_CACHE:
        _NC_CACHE["nc"] = build_kernel()
    return _NC_CACHE["nc"]


def kernel(**inputs):
    global LAST_RESULT
    graph = np.ascontiguousarray(np.asarray(inputs["input_graph"], np.float32))
    img = np.asarray(inputs["input_image"], np.float32).reshape(B, C, L)
    Wq = np.asarray(inputs["Wq"], np.float32)
    bq = np.asarray(inputs["bq"], np.float32)
    Wk = np.asarray(inputs["Wk"], np.float32)
    bk = np.asarray(inputs["bk"], np.float32)
    Wv = np.asarray(inputs["Wv"], np.float32)
    bv = np.asarray(inputs["bv"], np.float32)
    Wc = np.asarray(inputs["Wc"], np.float32)
    bc = np.asarray(inputs["bc"], np.float32)

    s = 1.0 / np.sqrt(np.float32(GC))

    # image: [B, 256, L] -> [B, 128, 2L] (channel halves side by side), bf16
    img_b = np.ascontiguousarray(
        img.reshape(B, 2, 128, L).transpose(0, 2, 1, 3).reshape(B, 128, 2 * L)
    ).astype(ml_dtypes.bfloat16)
    graphT = np.ascontiguousarray(graph.transpose(0, 2, 1))

    wkv = np.zeros((128, 160), np.float32)
    wkv[:, 0:32] = Wk.T[0:128]
    wkv[:, 32:64] = Wk.T[128:256]
    wkv[:, 64:96] = Wv.T[0:128]
    wkv[:, 96:128] = Wv.T[128:256]
    wkv[0:32, 128:160] = Wc.T
    wkv = wkv.astype(ml_dtypes.bfloat16)

    wq = np.zeros((GC, 40), np.float32)
    wq[:, 0:32] = Wq.T * s
    wq[:, 32] = bq * s
    wq[:, 33] = bk
    wq[:, 34] = bv
    wq[:, 35] = bc

    nc = _get_nc()
    in_maps = [
        {"img": img_b[i], "graphT": graphT[i], "wkv": wkv, "wq": wq}
        for i in range(B)
    ]
    res = run_bass_kernel_spmd(nc, in_maps, core_ids=list(range(B)),
                               trace=TRACE)
    LAST_RESULT = res
    outT = np.stack([np.asarray(res.results[i]["outT"]) for i in range(B)])
    return np.ascontiguousarray(outT.transpose(0, 2, 1)).astype(np.float32)


# revision 14
# speedup vs baseline: 1.1569x; 1.1569x over previous
"""Trainium2 Bass kernel for nn_ConnectionC2G (GNN cross-attention message passing).

Math (per batch b):
    K = Wk @ img + bk            [32, L]   (img = image reshaped [256, L], L = 4096)
    V = Wv @ img + bv            [32, L]
    Qt = (Wq @ graph^T + bq)/s   [32, N]   (s = sqrt(32); scale folded into Wq, bq)
    S^T[l, n] = sum_o K[o,l] Qt[o,n]       (attention scores, transposed layout)
    softmax over n-axis of the ORIGINAL layout == per-l-row softmax in S^T layout
    message[o, n] = sum_l (V[o,l]/den[l]) * exp(S^T[l,n])
    out^T = graph^T + Wc @ message + bc    [32, N]

Key tricks:
  - scores lie in [-2.6, 2.7] for this problem so exp() never overflows ->
    no max-subtraction pass; ScalarE activation computes exp straight from
    PSUM and its accum_out gives the softmax denominator for free.
  - 1/den is folded into V^T columns (per-partition scalar multiply) instead
    of normalizing the big [L, N] matrix.
  - message accumulates across all 32 l-tiles into 2 persistent PSUM banks
    using tile_position column strips (M=32 outputs packed 4-per-bank).
  - sharding: data-parallel over batch, 1 batch per NeuronCore (8 cores).

Host side pre-transposes graph -> graph^T, converts the image to bf16 in a
[128, 2*L] channel-split layout, packs the tiny weights, and transposes the
[32, N] device output back to [N, 32].
"""

import numpy as np
import ml_dtypes

import concourse.bass as bass
import concourse.bacc as bacc
import concourse.tile as tile
from concourse import mybir, masks
from concourse.bass_utils import run_bass_kernel_spmd

F32 = mybir.dt.float32
BF16 = mybir.dt.bfloat16
AF = mybir.ActivationFunctionType
OP = mybir.AluOpType

B = 8
N = 4096          # graph nodes
GC = 32           # graph channels
C = 256           # image channels
L = 4096          # image pixels (64*64)
LT = 128          # l-tile rows (partition dim of S^T tiles)
NLT = L // LT     # 32 l-tiles
NB = 512          # matmul moving-dim block
NNB = N // NB     # 8 n-blocks
# exp chunk boundaries within an l-tile's 4096 n-columns (3 PSUM banks each)
CHUNKS = [(0, 1536), (1536, 3072), (3072, 4096)]

TRACE = False            # test.py sets kernel.TRACE = True for profiling
LAST_RESULT = None       # test.py reads exec_time_ns from here

_NC_CACHE = {}


def build_kernel():
    nc = bacc.Bacc("TRN2")

    img_d = nc.dram_tensor("img", [128, 2 * L], BF16, kind="ExternalInput")
    graphT_d = nc.dram_tensor("graphT", [GC, N], F32, kind="ExternalInput")
    # bf16 pack: [:,0:32] WkT rows 0:128 | [:,32:64] WkT rows 128:256
    #            [:,64:96] WvT rows 0:128 | [:,96:128] WvT rows 128:256
    #            [0:32,128:160] WcT | [0:32,160:192] WqT*s
    wkv_d = nc.dram_tensor("wkv", [128, 192], BF16, kind="ExternalInput")
    graphTb_d = nc.dram_tensor("graphTb", [GC, N], BF16, kind="ExternalInput")
    # f32 pack: [:,0:32] WqT*s | [:,32] bq*s | [:,33] bk | [:,34] bv | [:,35] bc
    # row 0 cols 36:68 = bv again (free-dim copy for partition-broadcast DMA)
    wq_d = nc.dram_tensor("wq", [GC, 72], F32, kind="ExternalInput")
    out_d = nc.dram_tensor("outT", [GC, N], F32, kind="ExternalOutput")

    with tile.TileContext(nc) as tc:
        with tc.tile_pool(name="persist", bufs=1) as persist:
            img = persist.tile([128, 2 * L], BF16, tag="img")
            graphT = persist.tile([GC, N], F32, tag="graphT")
            graphTb = persist.tile([GC, N], BF16, tag="graphTb")
            wkv = persist.tile([128, 192], BF16, tag="wkv")
            wq = persist.tile([GC, 72], F32, tag="wq")
            bv_bcast = persist.tile([128, GC], F32, tag="bv_bcast")
            K_sb = persist.tile([GC, N], BF16, tag="K_sb")
            Qt = persist.tile([GC, N], BF16, tag="Qt")
            Vt_raw = persist.tile([128, NLT * GC], BF16, tag="Vt_raw")
            msg_sb = persist.tile([GC, N], BF16, tag="msg_sb")
            outT = persist.tile([GC, N], F32, tag="outT")

            # weights/graph first (small, unblock projections), image in l-halves
            # spread over several DMA queues so transfers overlap
            nc.scalar.dma_start(out=wkv[:], in_=wkv_d[:])
            nc.scalar.dma_start(out=wq[:], in_=wq_d[:])
            # bv broadcast to all partitions (stride-0 partition DMA)
            bv_row = wq_d[0:1, 36:68]
            nc.scalar.dma_start(
                out=bv_bcast[:],
                in_=bass.AP(tensor=bv_row.tensor, offset=bv_row.offset,
                            ap=[[0, 128]] + list(bv_row.ap[1:])))
            nc.scalar.dma_start(out=graphTb[:], in_=graphTb_d[:])
            nc.scalar.dma_start(out=graphT[:], in_=graphT_d[:])
            HL = 2048
            nc.sync.dma_start(out=img[:, 0:HL], in_=img_d[:, 0:HL])
            nc.sync.dma_start(out=img[:, L:L + HL], in_=img_d[:, L:L + HL])
            nc.gpsimd.dma_start(out=img[:, HL:L], in_=img_d[:, HL:L])
            nc.gpsimd.dma_start(out=img[:, L + HL:2 * L],
                                in_=img_d[:, L + HL:2 * L])

            bq = wq[:, 32:33]
            bk = wq[:, 33:34]
            bc = wq[:, 35:36]

            # ---- prologue: K/Q projections, then direct-V^T matmuls ------
            with (
                tc.tile_pool(name="proj_psum", bufs=3,
                             space=bass.MemorySpace.PSUM) as pp,
                tc.tile_pool(name="vt_psum", bufs=3,
                             space=bass.MemorySpace.PSUM) as vtp,
            ):
                for j in range(NNB):
                    blk = slice(j * NB, (j + 1) * NB)
                    kp = pp.tile([GC, NB], F32, tag="proj")
                    nc.tensor.matmul(kp[:], wkv[:, 0:32], img[:, blk],
                                     start=True, stop=False)
                    nc.tensor.matmul(kp[:], wkv[:, 32:64],
                                     img[:, L + j * NB:L + (j + 1) * NB],
                                     start=False, stop=True)
                    nc.vector.tensor_scalar_add(K_sb[:, blk], kp[:], bk)

                    qp = pp.tile([GC, NB], F32, tag="proj")
                    nc.tensor.matmul(qp[:], wkv[0:32, 160:192], graphTb[:, blk],
                                     start=True, stop=True)
                    nc.vector.tensor_scalar_add(Qt[:, blk], qp[:], bq)

                # V^T tiles directly: vt[l, o] = sum_c img[c, l] * WvT[c, o]
                # (img block is the stationary operand, no transpose pass)
                for lt in range(NLT):
                    vt = vtp.tile([128, GC], F32, tag="vt")
                    nc.tensor.matmul(vt[:], img[:, lt * LT:(lt + 1) * LT],
                                     wkv[:, 64:96], start=True, stop=False)
                    nc.tensor.matmul(vt[:],
                                     img[:, L + lt * LT:L + (lt + 1) * LT],
                                     wkv[:, 96:128], start=False, stop=True)
                    nc.vector.tensor_add(
                        Vt_raw[:, lt * GC:(lt + 1) * GC], vt[:], bv_bcast[:])

            # ---- main loop: scores -> exp -> message ---------------------
            with (
                tc.tile_pool(name="s_psum", bufs=2,
                             space=bass.MemorySpace.PSUM) as sp,
                tc.tile_pool(name="msg_psum", bufs=1,
                             space=bass.MemorySpace.PSUM) as mp,
                tc.tile_pool(name="e_pool", bufs=3) as ep,
                tc.tile_pool(name="stat", bufs=6) as stp,
            ):
                msg_ps = mp.tile([128, 1024], F32, tag="msg")
                prev = None  # (vts, e_t) of tile lt-1, msg emitted one behind

                def emit_msg(lt, vts, e_t):
                    for j in range(NNB):
                        cg = 32 * (j % 4)
                        hb = (j // 4) * NB
                        nc.tensor.matmul(
                            msg_ps[cg:cg + 32, hb:hb + NB],
                            vts[:], e_t[:, j * NB:(j + 1) * NB],
                            start=(lt == 0), stop=(lt == NLT - 1),
                            tile_position=(0, cg))

                for lt in range(NLT):
                    k_station = K_sb[:, lt * LT:(lt + 1) * LT]
                    e_t = ep.tile([128, N], BF16, tag="E")
                    accs = []
                    for (c0, c1) in CHUNKS:
                        w = c1 - c0
                        s_t = sp.tile([128, 1536], F32, tag="S")
                        for m in range(w // NB):
                            nc.tensor.matmul(
                                s_t[:, m * NB:(m + 1) * NB],
                                k_station,
                                Qt[:, c0 + m * NB:c0 + (m + 1) * NB],
                                start=True, stop=True)
                        acc = stp.tile([128, 1], F32, tag=f"acc{c0}")
                        nc.scalar.activation(
                            out=e_t[:, c0:c1], in_=s_t[:, 0:w],
                            func=AF.Exp, accum_out=acc[:])
                        accs.append(acc)
                    # message matmuls run one tile behind: their inputs are
                    # already ready, so the PE never waits on the den chain
                    if prev is not None:
                        emit_msg(lt - 1, *prev)
                    den = stp.tile([128, 1], F32, tag="den")
                    nc.vector.scalar_tensor_tensor(
                        out=den[:], in0=accs[0][:], scalar=accs[1][:],
                        in1=accs[2][:], op0=OP.add, op1=OP.add)
                    rden = stp.tile([128, 1], F32, tag="rden")
                    nc.vector.reciprocal(rden[:], den[:])
                    vts = stp.tile([128, GC], BF16, tag="vts")
                    nc.vector.tensor_scalar_mul(
                        vts[:], Vt_raw[:, lt * GC:(lt + 1) * GC], rden[:])
                    prev = (vts, e_t)
                emit_msg(NLT - 1, *prev)

                # unpack message strips to SBUF while pools still own psum
                for j in range(NNB):
                    cg = 32 * (j % 4)
                    hb = (j // 4) * NB
                    nc.vector.tensor_copy(
                        msg_sb[:, j * NB:(j + 1) * NB],
                        msg_ps[cg:cg + 32, hb:hb + NB])

            # ---- tail: Wc projection + residual --------------------------
            with tc.tile_pool(name="tail_psum", bufs=2,
                              space=bass.MemorySpace.PSUM) as tp:
                for j in range(NNB):
                    blk = slice(j * NB, (j + 1) * NB)
                    pj = tp.tile([GC, NB], F32, tag="prj")
                    nc.tensor.matmul(pj[:], wkv[0:32, 128:160], msg_sb[:, blk],
                                     start=True, stop=True)
                    nc.vector.scalar_tensor_tensor(
                        out=outT[:, blk], in0=pj[:], scalar=bc,
                        in1=graphT[:, blk], op0=OP.add, op1=OP.add)
                nc.sync.dma_start(out=out_d[:], in_=outT[:])

    nc.finalize()
    return nc


def _get_nc():
    if "nc" not in _NC_CACHE:
        _NC_CACHE["nc"] = build_kernel()
    return _NC_CACHE["nc"]


def kernel(**inputs):
    global LAST_RESULT
    graph = np.ascontiguousarray(np.asarray(inputs["input_graph"], np.float32))
    img = np.asarray(inputs["input_image"], np.float32).reshape(B, C, L)
    Wq = np.asarray(inputs["Wq"], np.float32)
    bq = np.asarray(inputs["bq"], np.float32)
    Wk = np.asarray(inputs["Wk"], np.float32)
    bk = np.asarray(inputs["bk"], np.float32)
    Wv = np.asarray(inputs["Wv"], np.float32)
    bv = np.asarray(inputs["bv"], np.float32)
    Wc = np.asarray(inputs["Wc"], np.float32)
    bc = np.asarray(inputs["bc"], np.float32)

    s = 1.0 / np.sqrt(np.float32(GC))

    # image: [B, 256, L] -> [B, 128, 2L] (channel halves side by side), bf16
    img_b = np.ascontiguousarray(
        img.reshape(B, 2, 128, L).transpose(0, 2, 1, 3).reshape(B, 128, 2 * L)
    ).astype(ml_dtypes.bfloat16)
    graphT = np.ascontiguousarray(graph.transpose(0, 2, 1))

    wkv = np.zeros((128, 192), np.float32)
    wkv[:, 0:32] = Wk.T[0:128]
    wkv[:, 32:64] = Wk.T[128:256]
    wkv[:, 64:96] = Wv.T[0:128]
    wkv[:, 96:128] = Wv.T[128:256]
    wkv[0:32, 128:160] = Wc.T
    wkv[0:32, 160:192] = Wq.T * s
    wkv = wkv.astype(ml_dtypes.bfloat16)

    wq = np.zeros((GC, 72), np.float32)
    wq[:, 0:32] = Wq.T * s
    wq[:, 32] = bq * s
    wq[:, 33] = bk
    wq[:, 34] = bv
    wq[:, 35] = bc
    wq[0, 36:68] = bv

    graphTb = graphT.astype(ml_dtypes.bfloat16)

    nc = _get_nc()
    in_maps = [
        {"img": img_b[i], "graphT": graphT[i], "graphTb": graphTb[i],
         "wkv": wkv, "wq": wq}
        for i in range(B)
    ]
    res = run_bass_kernel_spmd(nc, in_maps, core_ids=list(range(B)),
                               trace=TRACE)
    LAST_RESULT = res
    outT = np.stack([np.asarray(res.results[i]["outT"]) for i in range(B)])
    return np.ascontiguousarray(outT.transpose(0, 2, 1)).astype(np.float32)
